# revision 9
# baseline (speedup 1.0000x reference)
"""Trainium2 Bass kernel for nn_GATConv (gnn_message_passing).

Math (see reference):
    X' = X @ W                                     [N, OUT]
    f_e = <X'[row_e], X'[col_e]>                   per edge (uniform degree DEG CSR)
    out[r] = sum_{e in row r} (f_e * s) * X'[col_e],  s = sum(attention_w)

Distribution (8 NeuronCores, SPMD, one jitted program):
  - Rows sharded 8 ways. NEFF1: each core computes its X' shard from a
    host-pretransposed X^T shard (K-tiled PE matmul) and appends zero pad rows.
  - jax.lax.all_gather concatenates the 8 padded shards into the full banked
    neighbor table (device-resident, no host round trip). The gather custom op
    (dma_gather) and collectives cannot share a NEFF on this runtime, hence the
    split.
  - NEFF2: per row tile of 128 rows, neighbor rows are fetched with dma_gather
    (int16 bank-local indices; the table is split in 4 banks so indices fit
    int16; zero pad rows make slot padding contribute exactly 0). Edge features
    f and the attention-weighted aggregation run on DVE with fused
    scalar_tensor_tensor ops (multiply + free-dim accumulate), 2 elem/cycle.
  - Rows are sorted per-core by per-bank degree vector so the 128 rows of a
    tile need near-identical per-bank slot counts (minimal padding). The slot
    schedule is shared across cores (max over cores) so one program serves all.

kernel() takes full unsharded inputs and returns the full output.
"""
import os
import sys

sys.path.insert(0, "/opt/trn_rl_repo")

import numpy as np

import concourse.bacc as bacc
import concourse.bass as bass
import concourse.mybir as mybir
import concourse.tile as tile

F32 = mybir.dt.float32
I16 = mybir.dt.int16


class Cfg:
    def __init__(self, n_nodes=100_000, deg=16, in_dim=256, out_dim=128,
                 n_cores=8, group=3):
        assert n_nodes % n_cores == 0
        self.N = n_nodes
        self.DEG = deg
        self.IN = in_dim
        self.OUT = out_dim
        self.NC = n_cores
        self.RPC = n_nodes // n_cores                    # rows per core
        self.NT = (self.RPC + 127) // 128                # row tiles per core
        self.RPAD = self.NT * 128                        # padded rows per core
        # shard rows in the gathered table: RPC real + zero pad; covers RPAD
        self.SHARD = max(self.RPAD, self.RPC + 32)
        if self.SHARD == self.RPC:
            self.SHARD += 32
        self.NBANK = 4
        assert n_cores % self.NBANK == 0
        self.SPB = n_cores // self.NBANK                 # shards per bank
        self.BANKR = self.SPB * self.SHARD               # rows per bank
        assert self.BANKR - 1 < 2 ** 15, "bank too big for int16 idx"
        self.TABR = self.NC * self.SHARD                 # total table rows
        self.PADLOC = self.RPC                           # zero row, shard-local
        self.G = group                                   # tiles per gather group
        self.KT = in_dim // 128                          # k chunks in matmul
        assert in_dim % 128 == 0 and out_dim == 128


class Plan:
    """Host-derived, core-independent program structure."""
    def __init__(self, cfg, m):                          # m: [NT, NBANK] slots
        self.cfg = cfg
        self.m = m
        self.groups = [list(range(g, min(g + cfg.G, cfg.NT)))
                       for g in range(0, cfg.NT, cfg.G)]
        self.slotbase = {}               # (t,b) -> slot base within group tile
        self.g_slots = []                # slots per group
        self.idx_cols = {}               # (g,b) -> (col_off, n_idx)
        col = 0
        for gi, grp in enumerate(self.groups):
            s = 0
            for b in range(cfg.NBANK):
                for t in grp:
                    self.slotbase[(t, b)] = s
                    s += int(m[t, b])
            self.g_slots.append(s)
            for b in range(cfg.NBANK):
                n_idx = 128 * int(sum(m[t, b] for t in grp))
                self.idx_cols[(gi, b)] = (col, n_idx)
                col += n_idx // 16
        self.IC = col                    # gidx columns
        self.tile_slots = [[(b, self.slotbase[(t, b)] + j)
                            for b in range(cfg.NBANK)
                            for j in range(int(m[t, b]))]
                           for t in range(cfg.NT)]


def _wrap16(flat):
    """dma_gather index layout: idx i -> [i % 16, i // 16], replicated x8."""
    n = len(flat)
    w = np.zeros((16, n // 16), np.int16)
    w[np.arange(n) % 16, np.arange(n) // 16] = flat
    return np.tile(w, (8, 1))


def prep(cfg, column_index):
    """Returns (plan, per-core gidx arrays, per-core row perms)."""
    N, DEG, RPC, NC = cfg.N, cfg.DEG, cfg.RPC, cfg.NC
    cols = column_index.reshape(N, DEG).astype(np.int64)
    tshard = cols // RPC
    tbank = tshard // cfg.SPB

    nb = np.zeros((N, cfg.NBANK), np.int32)
    for b in range(cfg.NBANK):
        nb[:, b] = (tbank == b).sum(1)

    perms, pos_of = [], np.empty(N, np.int64)
    for c in range(NC):
        v = nb[c * RPC:(c + 1) * RPC]
        perm = np.lexsort(tuple(v[:, b] for b in range(cfg.NBANK - 1, -1, -1)))
        perms.append(perm)
        pos_of[c * RPC + perm] = np.arange(RPC)

    # bank-local index of each edge target (after target-shard row permutation)
    loc = (tshard % cfg.SPB) * cfg.SHARD + pos_of[cols]  # [N, DEG]

    # shared slot schedule: max per-bank count per tile over cores
    m = np.zeros((cfg.NT, cfg.NBANK), np.int32)
    core_sorted = []
    for c in range(NC):
        perm = perms[c]
        b_s = np.full((cfg.RPAD, DEG), -1, np.int64)
        l_s = np.zeros((cfg.RPAD, DEG), np.int64)
        b_s[:RPC] = tbank[c * RPC:][:RPC][perm]
        l_s[:RPC] = loc[c * RPC:][:RPC][perm]
        core_sorted.append((b_s, l_s))
        for b in range(cfg.NBANK):
            cnt = (b_s == b).sum(1).reshape(cfg.NT, 128)
            m[:, b] = np.maximum(m[:, b], cnt.max(1))

    plan = Plan(cfg, m)

    gidxs = []
    for c in range(NC):
        b_s, l_s = core_sorted[c]
        pieces = []
        for gi, grp in enumerate(plan.groups):
            for b in range(cfg.NBANK):
                chunk = []
                for t in grp:
                    mb = int(m[t, b])
                    if mb == 0:
                        continue
                    bb = b_s[t * 128:(t + 1) * 128]      # [128, DEG]
                    ll = l_s[t * 128:(t + 1) * 128]
                    mask = bb == b
                    order = np.argsort(~mask, 1, kind="stable")
                    lsel = np.take_along_axis(ll, order, 1)[:, :mb]
                    valid = np.take_along_axis(mask, order, 1)[:, :mb]
                    lsel = np.where(valid, lsel, cfg.PADLOC)  # zero row
                    chunk.append(lsel.T.ravel())         # s-major, then p
                if chunk:
                    pieces.append(_wrap16(np.concatenate(chunk).astype(np.int16)))
        gidxs.append(np.concatenate(pieces, 1) if pieces
                     else np.zeros((128, 0), np.int16))
    assert all(g.shape[1] == plan.IC for g in gidxs)
    return plan, gidxs, perms


def build_phase1(cfg):
    """NEFF1: X' shard = (X^T)^T @ W, plus zero pad rows. No custom ops."""
    nc = bacc.Bacc("TRN2", target_bir_lowering=False, debug=False,
                   enable_asserts=False, num_devices=cfg.NC)
    xt_in = nc.dram_tensor("xt_in", [cfg.IN, cfg.RPC], F32, kind="ExternalInput")
    w_in = nc.dram_tensor("w_in", [cfg.IN, cfg.OUT], F32, kind="ExternalInput")
    xps = nc.dram_tensor("xps", [cfg.SHARD, cfg.OUT], F32, kind="ExternalOutput")

    D = cfg.OUT
    with tile.TileContext(nc) as tc:
        with (
            tc.tile_pool(name="sb", bufs=3) as pool,
            tc.tile_pool(name="ps", bufs=4, space="PSUM") as psum,
        ):
            w_sb = pool.tile([128, cfg.KT, D], F32, bufs=1)
            nc.sync.dma_start(
                w_sb[:], w_in[:].rearrange("(k p) d -> p k d", p=128))
            for t in range(cfg.NT):
                r0 = t * 128
                rows = min(128, cfg.RPC - r0)
                xt_sb = pool.tile([128, cfg.KT, 128], F32)
                nc.sync.dma_start(
                    xt_sb[:, :, :rows],
                    xt_in[:, r0:r0 + rows].rearrange("(k p) r -> p k r", p=128))
                mm = psum.tile([128, D], F32)
                for k in range(cfg.KT):
                    nc.tensor.matmul(mm[:rows, :], xt_sb[:, k, :rows],
                                     w_sb[:, k, :],
                                     start=(k == 0), stop=(k == cfg.KT - 1))
                xp_sb = pool.tile([128, D], F32)
                nc.vector.tensor_copy(xp_sb[:rows, :], mm[:rows, :])
                nc.scalar.dma_start(xps[r0:r0 + rows, :], xp_sb[:rows, :])
            z_sb = pool.tile([128, D], F32, bufs=1)
            nc.vector.memset(z_sb[:], 0.0)
            r = cfg.RPC
            while r < cfg.SHARD:
                n = min(128, cfg.SHARD - r)
                nc.scalar.dma_start(xps[r:r + n, :], z_sb[:n, :])
                r += n
    nc.compile()
    return nc


def build_phase2(cfg, plan):
    """NEFF2: per-edge gather + attention + aggregation. No collectives."""
    nc = bacc.Bacc("TRN2", target_bir_lowering=False, debug=False,
                   enable_asserts=False, num_devices=cfg.NC,
                   dynamic_dma_scratch_size=2 ** 16)
    xtab = nc.dram_tensor("xtab", [cfg.TABR, cfg.OUT], F32, kind="ExternalInput")
    xps = nc.dram_tensor("xps2", [cfg.SHARD, cfg.OUT], F32, kind="ExternalInput")
    aw_in = nc.dram_tensor("aw_in", [1, 8], F32, kind="ExternalInput")
    gidx_in = nc.dram_tensor("gidx_in", [128, max(plan.IC, 16)], I16,
                             kind="ExternalInput")
    out = nc.dram_tensor("out", [cfg.RPAD, cfg.OUT], F32, kind="ExternalOutput")

    D = cfg.OUT
    with tile.TileContext(nc) as tc:
        with (
            tc.tile_pool(name="sb", bufs=2) as pool,
        ):
            aw_sb = pool.tile([128, 8], F32, bufs=1)
            nc.sync.dma_start(aw_sb[:], aw_in[0:1, :].to_broadcast([128, 8]))
            s_vec = pool.tile([128, 1], F32, bufs=1)
            nc.vector.reduce_sum(s_vec[:], aw_sb[:], axis=mybir.AxisListType.X)
            gidx_sb = pool.tile([128, max(plan.IC, 16)], I16, bufs=1)
            nc.sync.dma_start(gidx_sb[:], gidx_in[:])

            for gi, grp in enumerate(plan.groups):
                sg = plan.g_slots[gi]
                d_g = pool.tile([128, max(sg, 1) * D], F32)
                for b in range(cfg.NBANK):
                    col, n_idx = plan.idx_cols[(gi, b)]
                    if n_idx == 0:
                        continue
                    sbase = min(plan.slotbase[(t, b)] for t in grp)
                    nslots = n_idx // 128
                    nc.gpsimd.dma_gather(
                        out_ap=d_g[:, sbase * D:(sbase + nslots) * D]
                            .rearrange("p (s d) -> p s d", d=D),
                        in_ap=xtab[b * cfg.BANKR:(b + 1) * cfg.BANKR, :],
                        idxs_ap=gidx_sb[:, col:col + n_idx // 16],
                        num_idxs=n_idx,
                        num_idxs_reg=n_idx,
                        elem_size=D,
                        single_packet=False,
                    )
                for t in grp:
                    st = plan.tile_slots[t]
                    r0 = t * 128
                    r_sb = pool.tile([128, D], F32)
                    nc.sync.dma_start(r_sb[:], xps[r0:r0 + 128, :])
                    f_all = pool.tile([128, max(len(st), 1)], F32)
                    for si, (b, spos) in enumerate(st):
                        scr = pool.tile([128, D], F32, bufs=4)
                        nc.vector.scalar_tensor_tensor(
                            out=scr[:],
                            in0=d_g[:, spos * D:(spos + 1) * D],
                            scalar=s_vec[:, 0:1],
                            in1=r_sb[:],
                            op0=mybir.AluOpType.mult,
                            op1=mybir.AluOpType.mult,
                            accum_out=f_all[:, si:si + 1],
                        )
                    acc_a = pool.tile([128, D], F32)
                    acc_b = pool.tile([128, D], F32)
                    accs = [acc_a, acc_b]
                    if not st:
                        nc.vector.memset(acc_a[:], 0.0)
                        final = acc_a
                    else:
                        (b0, spos0) = st[0]
                        nc.vector.tensor_scalar(
                            out=acc_a[:], in0=d_g[:, spos0 * D:(spos0 + 1) * D],
                            scalar1=f_all[:, 0:1], scalar2=None,
                            op0=mybir.AluOpType.mult)
                        for si in range(1, len(st)):
                            (_b, spos) = st[si]
                            nc.vector.scalar_tensor_tensor(
                                out=accs[si % 2][:],
                                in0=d_g[:, spos * D:(spos + 1) * D],
                                scalar=f_all[:, si:si + 1],
                                in1=accs[(si + 1) % 2][:],
                                op0=mybir.AluOpType.mult,
                                op1=mybir.AluOpType.add,
                            )
                        final = accs[(len(st) - 1) % 2]
                    nc.scalar.dma_start(out[r0:r0 + 128, :], final[:])
    nc.compile()
    return nc


# ---------------------------------------------------------------------------
# jitted runner: NEFF1 -> lax.all_gather -> NEFF2, one program, jit once
# ---------------------------------------------------------------------------
def _neff_io(nc):
    part = nc.partition_id_tensor.name if nc.partition_id_tensor else None
    in_names, out_names, out_avals, zero_outs = [], [], [], []
    import jax
    for alloc in nc.m.functions[0].allocations:
        if not isinstance(alloc, mybir.MemoryLocationSet):
            continue
        name = alloc.memorylocations[0].name
        if alloc.kind == "ExternalInput":
            if name != part:
                in_names.append(name)
        elif alloc.kind == "ExternalOutput":
            out_names.append(name)
            shape = tuple(alloc.tensor_shape)
            dtype = mybir.dt.np(alloc.dtype)
            out_avals.append(jax.core.ShapedArray(shape, dtype))
            zero_outs.append(np.zeros(shape, dtype))
    return part, in_names, out_names, out_avals, zero_outs


class Runner:
    """Three device-resident stages: NEFF1 -> XLA all_gather -> NEFF2.

    The neuronx_cc_hook only accepts modules that are exactly one bass_exec
    custom-call over the jit parameters, so each bass NEFF is its own jit and
    the all-gather is a separate stock-XLA jit. jax arrays stay on device
    between the three dispatches.
    """
    def __init__(self, cfg, nc1, nc2):
        import jax
        from jax.sharding import Mesh, PartitionSpec
        from jax.experimental.shard_map import shard_map
        from concourse import bass2jax
        bass2jax.install_neuronx_cc_hook()
        self.cfg = cfg
        self.jax = jax

        p1, in1, out1, av1, z1 = _neff_io(nc1)
        p2, in2, out2, av2, z2 = _neff_io(nc2)
        assert in1 == ["xt_in", "w_in"] and out1 == ["xps"], (in1, out1)
        assert in2 == ["xtab", "xps2", "aw_in", "gidx_in"] and out2 == ["out"]
        self.z1, self.z2 = z1, z2

        def bexec(nc, part, in_names, out_names, out_avals, *args):
            operands = list(args)
            if part is not None:
                operands.append(bass2jax.partition_id_tensor())
            return bass2jax._bass_exec_p.bind(
                *operands,
                out_avals=tuple(out_avals),
                in_names=tuple(in_names + out_names +
                               ([part] if part else [])),
                out_names=tuple(out_names),
                lowering_input_output_aliases=(),
                sim_require_finite=True,
                sim_require_nnan=True,
                nc=nc,
            )

        devices = jax.devices()[:cfg.NC]
        mesh = Mesh(np.asarray(devices), ("core",))
        P = PartitionSpec

        def _b1(xt, w, zxps):
            return tuple(bexec(nc1, p1, in1, out1, av1, xt, w, zxps))

        def _ag(xps):
            return (jax.lax.all_gather(xps, "core", axis=0, tiled=True),)

        def _b2(xtab, xps, aw, gidx, zout):
            return tuple(bexec(nc2, p2, in2, out2, av2, xtab, xps, aw, gidx,
                               zout))

        self._fn1 = jax.jit(
            shard_map(_b1, mesh=mesh, in_specs=(P("core"),) * 3,
                      out_specs=(P("core"),), check_rep=False),
            donate_argnums=(2,), keep_unused=True)
        self._fag = jax.jit(
            shard_map(_ag, mesh=mesh, in_specs=(P("core"),),
                      out_specs=(P("core"),), check_rep=False))
        self._fn2 = jax.jit(
            shard_map(_b2, mesh=mesh, in_specs=(P("core"),) * 5,
                      out_specs=(P("core"),), check_rep=False),
            donate_argnums=(4,), keep_unused=True)

    def run_stages(self, xts, w, aw, gidxs):
        n = self.cfg.NC
        cat = np.concatenate
        zxps = np.zeros((n * self.z1[0].shape[0], *self.z1[0].shape[1:]),
                        self.z1[0].dtype)
        zout = np.zeros((n * self.z2[0].shape[0], *self.z2[0].shape[1:]),
                        self.z2[0].dtype)
        (xps,) = self._fn1(cat(xts, 0), cat([w] * n, 0), zxps)
        (xtab,) = self._fag(xps)
        (o,) = self._fn2(xtab, xps, cat([aw] * n, 0), cat(gidxs, 0), zout)
        return o

    def __call__(self, xts, w, aw, gidxs):
        o = self.run_stages(xts, w, aw, gidxs)
        return np.asarray(o).reshape(self.cfg.NC, -1, self.cfg.OUT)


_CACHE = {}


def _get_runner(cfg, column_index):
    key = (cfg.N, cfg.DEG, cfg.IN, cfg.OUT, hash(column_index.tobytes()))
    if key not in _CACHE:
        plan, gidxs, perms = prep(cfg, column_index)
        pad = np.mean(plan.m.sum(1)) / cfg.DEG - 1.0
        print(f"[kernel] slot padding overhead: {pad * 100:.1f}%  "
              f"(avg slots/tile {plan.m.sum(1).mean():.1f})", file=sys.stderr)
        nc1 = build_phase1(cfg)
        nc2 = build_phase2(cfg, plan)
        runner = Runner(cfg, nc1, nc2)
        _CACHE[key] = (plan, gidxs, perms, runner)
    return _CACHE[key]


def _kernel_impl(cfg, X, weights, attention_w, column_index):
    plan, gidxs, perms, runner = _get_runner(cfg, column_index)
    aw = np.asarray(attention_w, np.float32).reshape(1, -1)
    if aw.shape[1] != 8:
        a8 = np.zeros((1, 8), np.float32)
        a8[0, :aw.shape[1]] = aw
        aw = a8
    xts, gis = [], []
    for c in range(cfg.NC):
        xs = X[c * cfg.RPC:(c + 1) * cfg.RPC][perms[c]]     # sorted rows
        xts.append(np.ascontiguousarray(xs.T, np.float32))
        gi = gidxs[c]
        if gi.shape[1] < 16:
            gi = np.zeros((128, 16), np.int16)
        gis.append(gi)
    o = runner(xts, np.asarray(weights, np.float32), aw, gis)
    out = np.empty((cfg.N, cfg.OUT), np.float32)
    for c in range(cfg.NC):
        out[c * cfg.RPC + perms[c]] = o[c][:cfg.RPC]
    return out


def kernel(X, weights, attention_w, row_pointers, column_index,
           blockPartition=None, edgeToColumn=None, edgeToRow=None):
    X = np.asarray(X)
    weights = np.asarray(weights)
    attention_w = np.asarray(attention_w)
    row_pointers = np.asarray(row_pointers)
    column_index = np.asarray(column_index)
    n, in_dim = X.shape
    out_dim = weights.shape[1]
    deg = column_index.shape[0] // n

    uniform = np.array_equal(
        row_pointers.astype(np.int64),
        np.arange(n + 1, dtype=np.int64) * deg)
    if not uniform or n % 8 != 0 or in_dim % 128 != 0 or out_dim != 128:
        # general CSR fallback (host, exact semantics of the reference)
        xp = X.astype(np.float64) @ weights.astype(np.float64)
        e = column_index.shape[0]
        row_ids = np.searchsorted(row_pointers, np.arange(e), side="right") - 1
        dst = xp[column_index]
        f = (xp[row_ids] * dst).sum(-1) * attention_w.sum()
        out = np.zeros((n, out_dim))
        np.add.at(out, row_ids, f[:, None] * dst)
        return out.astype(np.float32)

    cfg = Cfg(n_nodes=n, deg=deg, in_dim=in_dim, out_dim=out_dim)
    return _kernel_impl(cfg, np.asarray(X, np.float32),
                        weights, attention_w, column_index)


# revision 10
# speedup vs baseline: 29.4959x; 29.4959x over previous
"""Trainium2 Bass kernel for nn_GATConv (gnn_message_passing).

Math (see reference):
    X' = X @ W                                     [N, OUT]
    f_e = <X'[row_e], X'[col_e]>                   per edge (uniform degree DEG CSR)
    out[r] = sum_{e in row r} (f_e * s) * X'[col_e],  s = sum(attention_w)

Distribution (8 NeuronCores, SPMD, one jitted program):
  - Rows sharded 8 ways. NEFF1: each core computes its X' shard from a
    host-pretransposed X^T shard (K-tiled PE matmul) and appends zero pad rows.
  - jax.lax.all_gather concatenates the 8 padded shards into the full banked
    neighbor table (device-resident, no host round trip). The gather custom op
    (dma_gather) and collectives cannot share a NEFF on this runtime, hence the
    split.
  - NEFF2: per row tile of 128 rows, neighbor rows are fetched with dma_gather
    (int16 bank-local indices; the table is split in 4 banks so indices fit
    int16; zero pad rows make slot padding contribute exactly 0). Edge features
    f and the attention-weighted aggregation run on DVE with fused
    scalar_tensor_tensor ops (multiply + free-dim accumulate), 2 elem/cycle.
  - Rows are sorted per-core by per-bank degree vector so the 128 rows of a
    tile need near-identical per-bank slot counts (minimal padding). The slot
    schedule is shared across cores (max over cores) so one program serves all.

kernel() takes full unsharded inputs and returns the full output.
"""
import os
import sys

sys.path.insert(0, "/opt/trn_rl_repo")

import numpy as np

import concourse.bacc as bacc
import concourse.bass as bass
import concourse.mybir as mybir
import concourse.tile as tile

F32 = mybir.dt.float32
I16 = mybir.dt.int16


class Cfg:
    def __init__(self, n_nodes=100_000, deg=16, in_dim=256, out_dim=128,
                 n_cores=8, group=3):
        assert n_nodes % n_cores == 0
        self.N = n_nodes
        self.DEG = deg
        self.IN = in_dim
        self.OUT = out_dim
        self.NC = n_cores
        self.RPC = n_nodes // n_cores                    # rows per core
        self.NT = (self.RPC + 127) // 128                # row tiles per core
        self.RPAD = self.NT * 128                        # padded rows per core
        # shard rows in the gathered table: RPC real + zero pad; covers RPAD
        self.SHARD = max(self.RPAD, self.RPC + 32)
        if self.SHARD == self.RPC:
            self.SHARD += 32
        self.NBANK = 4
        assert n_cores % self.NBANK == 0
        self.SPB = n_cores // self.NBANK                 # shards per bank
        self.BANKR = self.SPB * self.SHARD               # rows per bank
        assert self.BANKR - 1 < 2 ** 15, "bank too big for int16 idx"
        self.TABR = self.NC * self.SHARD                 # total table rows
        self.PADLOC = self.RPC                           # zero row, shard-local
        self.G = group                                   # tiles per gather group
        self.KT = in_dim // 128                          # k chunks in matmul
        assert in_dim % 128 == 0 and out_dim == 128


class Plan:
    """Host-derived, core-independent program structure."""
    def __init__(self, cfg, m):                          # m: [NT, NBANK] slots
        self.cfg = cfg
        self.m = m
        self.groups = [list(range(g, min(g + cfg.G, cfg.NT)))
                       for g in range(0, cfg.NT, cfg.G)]
        self.slotbase = {}               # (t,b) -> slot base within group tile
        self.g_slots = []                # slots per group
        self.idx_cols = {}               # (g,b) -> (col_off, n_idx)
        col = 0
        for gi, grp in enumerate(self.groups):
            s = 0
            for b in range(cfg.NBANK):
                for t in grp:
                    self.slotbase[(t, b)] = s
                    s += int(m[t, b])
            self.g_slots.append(s)
            for b in range(cfg.NBANK):
                n_idx = 128 * int(sum(m[t, b] for t in grp))
                self.idx_cols[(gi, b)] = (col, n_idx)
                col += n_idx // 16
        self.IC = col                    # gidx columns
        self.tile_slots = [[(b, self.slotbase[(t, b)] + j)
                            for b in range(cfg.NBANK)
                            for j in range(int(m[t, b]))]
                           for t in range(cfg.NT)]


def _wrap16(flat):
    """dma_gather index layout: idx i -> [i % 16, i // 16], replicated x8."""
    n = len(flat)
    w = np.zeros((16, n // 16), np.int16)
    w[np.arange(n) % 16, np.arange(n) // 16] = flat
    return np.tile(w, (8, 1))


def prep(cfg, column_index):
    """Returns (plan, per-core gidx arrays, per-core row perms)."""
    N, DEG, RPC, NC = cfg.N, cfg.DEG, cfg.RPC, cfg.NC
    cols = column_index.reshape(N, DEG).astype(np.int64)
    tshard = cols // RPC
    tbank = tshard // cfg.SPB

    nb = np.zeros((N, cfg.NBANK), np.int32)
    for b in range(cfg.NBANK):
        nb[:, b] = (tbank == b).sum(1)

    perms, pos_of = [], np.empty(N, np.int64)
    for c in range(NC):
        v = nb[c * RPC:(c + 1) * RPC]
        perm = np.lexsort(tuple(v[:, b] for b in range(cfg.NBANK - 1, -1, -1)))
        perms.append(perm)
        pos_of[c * RPC + perm] = np.arange(RPC)

    # bank-local index of each edge target (after target-shard row permutation)
    loc = (tshard % cfg.SPB) * cfg.SHARD + pos_of[cols]  # [N, DEG]

    # shared slot schedule: max per-bank count per tile over cores
    m = np.zeros((cfg.NT, cfg.NBANK), np.int32)
    core_sorted = []
    for c in range(NC):
        perm = perms[c]
        b_s = np.full((cfg.RPAD, DEG), -1, np.int64)
        l_s = np.zeros((cfg.RPAD, DEG), np.int64)
        b_s[:RPC] = tbank[c * RPC:][:RPC][perm]
        l_s[:RPC] = loc[c * RPC:][:RPC][perm]
        core_sorted.append((b_s, l_s))
        for b in range(cfg.NBANK):
            cnt = (b_s == b).sum(1).reshape(cfg.NT, 128)
            m[:, b] = np.maximum(m[:, b], cnt.max(1))

    plan = Plan(cfg, m)

    gidxs = []
    for c in range(NC):
        b_s, l_s = core_sorted[c]
        pieces = []
        for gi, grp in enumerate(plan.groups):
            for b in range(cfg.NBANK):
                chunk = []
                for t in grp:
                    mb = int(m[t, b])
                    if mb == 0:
                        continue
                    bb = b_s[t * 128:(t + 1) * 128]      # [128, DEG]
                    ll = l_s[t * 128:(t + 1) * 128]
                    mask = bb == b
                    order = np.argsort(~mask, 1, kind="stable")
                    lsel = np.take_along_axis(ll, order, 1)[:, :mb]
                    valid = np.take_along_axis(mask, order, 1)[:, :mb]
                    lsel = np.where(valid, lsel, cfg.PADLOC)  # zero row
                    chunk.append(lsel.T.ravel())         # s-major, then p
                if chunk:
                    pieces.append(_wrap16(np.concatenate(chunk).astype(np.int16)))
        gidxs.append(np.concatenate(pieces, 1) if pieces
                     else np.zeros((128, 0), np.int16))
    assert all(g.shape[1] == plan.IC for g in gidxs)
    return plan, gidxs, perms


def build_phase1(cfg):
    """NEFF1: X' shard = (X^T)^T @ W, plus zero pad rows. No custom ops."""
    nc = bacc.Bacc("TRN2", target_bir_lowering=False, debug=False,
                   enable_asserts=False, num_devices=cfg.NC)
    xt_in = nc.dram_tensor("xt_in", [cfg.IN, cfg.RPC], F32, kind="ExternalInput")
    w_in = nc.dram_tensor("w_in", [cfg.IN, cfg.OUT], F32, kind="ExternalInput")
    xps = nc.dram_tensor("xps", [cfg.SHARD, cfg.OUT], F32, kind="ExternalOutput")

    D = cfg.OUT
    with tile.TileContext(nc) as tc:
        with (
            tc.tile_pool(name="sb", bufs=3) as pool,
            tc.tile_pool(name="ps", bufs=4, space="PSUM") as psum,
        ):
            w_sb = pool.tile([128, cfg.KT, D], F32, bufs=1)
            nc.sync.dma_start(
                w_sb[:], w_in[:].rearrange("(k p) d -> p k d", p=128))
            for t in range(cfg.NT):
                r0 = t * 128
                rows = min(128, cfg.RPC - r0)
                xt_sb = pool.tile([128, cfg.KT, 128], F32)
                nc.sync.dma_start(
                    xt_sb[:, :, :rows],
                    xt_in[:, r0:r0 + rows].rearrange("(k p) r -> p k r", p=128))
                mm = psum.tile([128, D], F32)
                for k in range(cfg.KT):
                    nc.tensor.matmul(mm[:rows, :], xt_sb[:, k, :rows],
                                     w_sb[:, k, :],
                                     start=(k == 0), stop=(k == cfg.KT - 1))
                xp_sb = pool.tile([128, D], F32)
                nc.vector.tensor_copy(xp_sb[:rows, :], mm[:rows, :])
                nc.scalar.dma_start(xps[r0:r0 + rows, :], xp_sb[:rows, :])
            z_sb = pool.tile([128, D], F32, bufs=1)
            nc.vector.memset(z_sb[:], 0.0)
            r = cfg.RPC
            while r < cfg.SHARD:
                n = min(128, cfg.SHARD - r)
                nc.scalar.dma_start(xps[r:r + n, :], z_sb[:n, :])
                r += n
    nc.compile()
    return nc


def build_phase2(cfg, plan):
    """NEFF2: per-edge gather + attention + aggregation. No collectives."""
    nc = bacc.Bacc("TRN2", target_bir_lowering=False, debug=False,
                   enable_asserts=False, num_devices=cfg.NC,
                   dynamic_dma_scratch_size=2 ** 16)
    xtab = nc.dram_tensor("xtab", [cfg.TABR, cfg.OUT], F32, kind="ExternalInput")
    xps = nc.dram_tensor("xps2", [cfg.SHARD, cfg.OUT], F32, kind="ExternalInput")
    aw_in = nc.dram_tensor("aw_in", [1, 8], F32, kind="ExternalInput")
    gidx_in = nc.dram_tensor("gidx_in", [128, max(plan.IC, 16)], I16,
                             kind="ExternalInput")
    out = nc.dram_tensor("out", [cfg.RPAD, cfg.OUT], F32, kind="ExternalOutput")

    D = cfg.OUT
    with tile.TileContext(nc) as tc:
        with (
            tc.tile_pool(name="sb", bufs=2) as pool,
        ):
            aw_sb = pool.tile([128, 8], F32, bufs=1)
            nc.sync.dma_start(aw_sb[:], aw_in[0:1, :].to_broadcast([128, 8]))
            s_vec = pool.tile([128, 1], F32, bufs=1)
            nc.vector.reduce_sum(s_vec[:], aw_sb[:], axis=mybir.AxisListType.X)
            gidx_sb = pool.tile([128, max(plan.IC, 16)], I16, bufs=1)
            nc.sync.dma_start(gidx_sb[:], gidx_in[:])

            for gi, grp in enumerate(plan.groups):
                sg = plan.g_slots[gi]
                d_g = pool.tile([128, max(sg, 1) * D], F32)
                for b in range(cfg.NBANK):
                    col, n_idx = plan.idx_cols[(gi, b)]
                    if n_idx == 0:
                        continue
                    sbase = min(plan.slotbase[(t, b)] for t in grp)
                    nslots = n_idx // 128
                    nc.gpsimd.dma_gather(
                        out_ap=d_g[:, sbase * D:(sbase + nslots) * D]
                            .rearrange("p (s d) -> p s d", d=D),
                        in_ap=xtab[b * cfg.BANKR:(b + 1) * cfg.BANKR, :],
                        idxs_ap=gidx_sb[:, col:col + n_idx // 16],
                        num_idxs=n_idx,
                        num_idxs_reg=n_idx,
                        elem_size=D,
                        single_packet=False,
                    )
                for t in grp:
                    st = plan.tile_slots[t]
                    r0 = t * 128
                    r_sb = pool.tile([128, D], F32)
                    nc.sync.dma_start(r_sb[:], xps[r0:r0 + 128, :])
                    f_all = pool.tile([128, max(len(st), 1)], F32)
                    for si, (b, spos) in enumerate(st):
                        scr = pool.tile([128, D], F32, bufs=4)
                        nc.vector.scalar_tensor_tensor(
                            out=scr[:],
                            in0=d_g[:, spos * D:(spos + 1) * D],
                            scalar=s_vec[:, 0:1],
                            in1=r_sb[:],
                            op0=mybir.AluOpType.mult,
                            op1=mybir.AluOpType.mult,
                            accum_out=f_all[:, si:si + 1],
                        )
                    acc_a = pool.tile([128, D], F32)
                    acc_b = pool.tile([128, D], F32)
                    accs = [acc_a, acc_b]
                    if not st:
                        nc.vector.memset(acc_a[:], 0.0)
                        final = acc_a
                    else:
                        (b0, spos0) = st[0]
                        nc.vector.tensor_scalar(
                            out=acc_a[:], in0=d_g[:, spos0 * D:(spos0 + 1) * D],
                            scalar1=f_all[:, 0:1], scalar2=None,
                            op0=mybir.AluOpType.mult)
                        for si in range(1, len(st)):
                            (_b, spos) = st[si]
                            nc.vector.scalar_tensor_tensor(
                                out=accs[si % 2][:],
                                in0=d_g[:, spos * D:(spos + 1) * D],
                                scalar=f_all[:, si:si + 1],
                                in1=accs[(si + 1) % 2][:],
                                op0=mybir.AluOpType.mult,
                                op1=mybir.AluOpType.add,
                            )
                        final = accs[(len(st) - 1) % 2]
                    nc.scalar.dma_start(out[r0:r0 + 128, :], final[:])
    nc.compile()
    return nc


# ---------------------------------------------------------------------------
# jitted runner: NEFF1 -> lax.all_gather -> NEFF2, one program, jit once
# ---------------------------------------------------------------------------
def _neff_io(nc):
    part = nc.partition_id_tensor.name if nc.partition_id_tensor else None
    in_names, out_names, out_avals, zero_outs = [], [], [], []
    import jax
    for alloc in nc.m.functions[0].allocations:
        if not isinstance(alloc, mybir.MemoryLocationSet):
            continue
        name = alloc.memorylocations[0].name
        if alloc.kind == "ExternalInput":
            if name != part:
                in_names.append(name)
        elif alloc.kind == "ExternalOutput":
            out_names.append(name)
            shape = tuple(alloc.tensor_shape)
            dtype = mybir.dt.np(alloc.dtype)
            out_avals.append(jax.core.ShapedArray(shape, dtype))
            zero_outs.append(np.zeros(shape, dtype))
    return part, in_names, out_names, out_avals, zero_outs


class Runner:
    """Three device-resident stages: NEFF1 -> XLA all_gather -> NEFF2.

    The neuronx_cc_hook only accepts modules that are exactly one bass_exec
    custom-call over the jit parameters, so each bass NEFF is its own jit and
    the all-gather is a separate stock-XLA jit. jax arrays stay on device
    between the three dispatches.
    """
    def __init__(self, cfg, nc1, nc2):
        import jax
        from jax.sharding import Mesh, PartitionSpec
        from jax.experimental.shard_map import shard_map
        from concourse import bass2jax
        bass2jax.install_neuronx_cc_hook()
        self.cfg = cfg
        self.jax = jax

        p1, in1, out1, av1, z1 = _neff_io(nc1)
        p2, in2, out2, av2, z2 = _neff_io(nc2)
        assert in1 == ["xt_in", "w_in"] and out1 == ["xps"], (in1, out1)
        assert in2 == ["xtab", "xps2", "aw_in", "gidx_in"] and out2 == ["out"]
        self.z1, self.z2 = z1, z2

        def bexec(nc, part, in_names, out_names, out_avals, *args):
            operands = list(args)
            if part is not None:
                operands.append(bass2jax.partition_id_tensor())
            return bass2jax._bass_exec_p.bind(
                *operands,
                out_avals=tuple(out_avals),
                in_names=tuple(in_names + out_names +
                               ([part] if part else [])),
                out_names=tuple(out_names),
                lowering_input_output_aliases=(),
                sim_require_finite=True,
                sim_require_nnan=True,
                nc=nc,
            )

        devices = jax.devices()[:cfg.NC]
        mesh = Mesh(np.asarray(devices), ("core",))
        self.mesh = mesh
        P = PartitionSpec

        def _b1(xt, w, zxps):
            return tuple(bexec(nc1, p1, in1, out1, av1, xt, w, zxps))

        def _ag(xps):
            return (jax.lax.all_gather(xps, "core", axis=0, tiled=True),)

        def _b2(xtab, xps, aw, gidx, zout):
            return tuple(bexec(nc2, p2, in2, out2, av2, xtab, xps, aw, gidx,
                               zout))

        self._fn1 = jax.jit(
            shard_map(_b1, mesh=mesh, in_specs=(P("core"),) * 3,
                      out_specs=(P("core"),), check_rep=False),
            donate_argnums=(2,), keep_unused=True)
        self._fag = jax.jit(
            shard_map(_ag, mesh=mesh, in_specs=(P("core"),),
                      out_specs=(P("core"),), check_rep=False))
        self._fn2 = jax.jit(
            shard_map(_b2, mesh=mesh, in_specs=(P("core"),) * 5,
                      out_specs=(P("core"),), check_rep=False),
            donate_argnums=(4,), keep_unused=True)

    def run_stages(self, xts, w, aw, gidxs):
        n = self.cfg.NC
        cat = np.concatenate
        zxps = np.zeros((n * self.z1[0].shape[0], *self.z1[0].shape[1:]),
                        self.z1[0].dtype)
        zout = np.zeros((n * self.z2[0].shape[0], *self.z2[0].shape[1:]),
                        self.z2[0].dtype)
        (xps,) = self._fn1(cat(xts, 0), cat([w] * n, 0), zxps)
        (xtab,) = self._fag(xps)
        (o,) = self._fn2(xtab, xps, cat([aw] * n, 0), cat(gidxs, 0), zout)
        return o

    def __call__(self, xts, w, aw, gidxs):
        o = self.run_stages(xts, w, aw, gidxs)
        return np.asarray(o).reshape(self.cfg.NC, -1, self.cfg.OUT)


_CACHE = {}


def _get_runner(cfg, column_index):
    key = (cfg.N, cfg.DEG, cfg.IN, cfg.OUT, hash(column_index.tobytes()))
    if key not in _CACHE:
        plan, gidxs, perms = prep(cfg, column_index)
        pad = np.mean(plan.m.sum(1)) / cfg.DEG - 1.0
        print(f"[kernel] slot padding overhead: {pad * 100:.1f}%  "
              f"(avg slots/tile {plan.m.sum(1).mean():.1f})", file=sys.stderr)
        nc1 = build_phase1(cfg)
        nc2 = build_phase2(cfg, plan)
        runner = Runner(cfg, nc1, nc2)
        _CACHE[key] = (plan, gidxs, perms, runner)
    return _CACHE[key]


def _kernel_impl(cfg, X, weights, attention_w, column_index):
    plan, gidxs, perms, runner = _get_runner(cfg, column_index)
    aw = np.asarray(attention_w, np.float32).reshape(1, -1)
    if aw.shape[1] != 8:
        a8 = np.zeros((1, 8), np.float32)
        a8[0, :aw.shape[1]] = aw
        aw = a8
    xts, gis = [], []
    for c in range(cfg.NC):
        xs = X[c * cfg.RPC:(c + 1) * cfg.RPC][perms[c]]     # sorted rows
        xts.append(np.ascontiguousarray(xs.T, np.float32))
        gi = gidxs[c]
        if gi.shape[1] < 16:
            gi = np.zeros((128, 16), np.int16)
        gis.append(gi)
    o = runner(xts, np.asarray(weights, np.float32), aw, gis)
    out = np.empty((cfg.N, cfg.OUT), np.float32)
    for c in range(cfg.NC):
        out[c * cfg.RPC + perms[c]] = o[c][:cfg.RPC]
    return out


def kernel(X, weights, attention_w, row_pointers, column_index,
           blockPartition=None, edgeToColumn=None, edgeToRow=None):
    X = np.asarray(X)
    weights = np.asarray(weights)
    attention_w = np.asarray(attention_w)
    row_pointers = np.asarray(row_pointers)
    column_index = np.asarray(column_index)
    n, in_dim = X.shape
    out_dim = weights.shape[1]
    deg = column_index.shape[0] // n

    uniform = np.array_equal(
        row_pointers.astype(np.int64),
        np.arange(n + 1, dtype=np.int64) * deg)
    if not uniform or n % 8 != 0 or in_dim % 128 != 0 or out_dim != 128:
        # general CSR fallback (host, exact semantics of the reference)
        xp = X.astype(np.float64) @ weights.astype(np.float64)
        e = column_index.shape[0]
        row_ids = np.searchsorted(row_pointers, np.arange(e), side="right") - 1
        dst = xp[column_index]
        f = (xp[row_ids] * dst).sum(-1) * attention_w.sum()
        out = np.zeros((n, out_dim))
        np.add.at(out, row_ids, f[:, None] * dst)
        return out.astype(np.float32)

    cfg = Cfg(n_nodes=n, deg=deg, in_dim=in_dim, out_dim=out_dim)
    return _kernel_impl(cfg, np.asarray(X, np.float32),
                        weights, attention_w, column_index)
